# revision 1
# baseline (speedup 1.0000x reference)
"""Trainium2 Bass kernel for nn_CoreDiffusion (gnn_message_passing).

Sharding: node dim N=4096 split across 8 cores (512 nodes each). Each core:
  msg[b,c] = adj[b,c,rows,:] @ x[b]   (fp16 operands, fp32 PSUM accum)
  hx[c] = relu(cumsum_c msg)          (fp32)
  GRU over c (float32r matmuls), sum over c, LayerNorm (fp32).
No collectives; full output gathered on host.
"""
import numpy as np
from contextlib import ExitStack

import concourse.bass as bass
import concourse.mybir as mybir
import concourse.tile as tile
from concourse import bacc
from concourse.masks import make_identity
from concourse.bass_utils import run_bass_kernel_spmd

F32 = mybir.dt.float32
F32R = mybir.dt.float32r
F16 = mybir.dt.float16
AF = mybir.ActivationFunctionType

B, C, N, D, H = 2, 4, 4096, 64, 64
NCORES = 8
NS = N // NCORES            # 512 nodes per core
JC = N // 128               # 32 contraction chunks
LN_EPS = 1e-5


def build():
    nc = bacc.Bacc("TRN2", target_bir_lowering=False, debug=False,
                   num_devices=NCORES)
    adj_s = nc.declare_dram_parameter("adj_s", [B, C, NS, N], F32, isOutput=False)
    x = nc.declare_dram_parameter("x", [B, N, D], F32, isOutput=False)
    w_ih = nc.declare_dram_parameter("w_ih", [3 * H, D], F32, isOutput=False)
    w_hh = nc.declare_dram_parameter("w_hh", [3 * H, H], F32, isOutput=False)
    b_ih = nc.declare_dram_parameter("b_ih", [3 * H], F32, isOutput=False)
    b_hh = nc.declare_dram_parameter("b_hh", [3 * H], F32, isOutput=False)
    gamma = nc.declare_dram_parameter("gamma", [H], F32, isOutput=False)
    beta = nc.declare_dram_parameter("beta", [H], F32, isOutput=False)
    out_s = nc.declare_dram_parameter("out_s", [B, NS, H], F32, isOutput=True)

    with tile.TileContext(nc) as tc, ExitStack() as ctx:
        const = ctx.enter_context(tc.tile_pool(name="const", bufs=1))
        adj_pool = ctx.enter_context(tc.tile_pool(name="adj", bufs=6))
        adjt_pool = ctx.enter_context(tc.tile_pool(name="adjt", bufs=6))
        gru = ctx.enter_context(tc.tile_pool(name="gru", bufs=2))
        psum = ctx.enter_context(tc.tile_pool(name="psum", bufs=1, space="PSUM"))
        psum_t = ctx.enter_context(tc.tile_pool(name="psum_t", bufs=3, space="PSUM"))
        psum_a = ctx.enter_context(tc.tile_pool(name="psum_a", bufs=1, space="PSUM"))

        # ---------- setup ----------
        ident = const.tile([128, 128], F32)
        make_identity(nc, ident)
        ident16 = const.tile([128, 128], F16)
        nc.vector.tensor_copy(ident16, ident)

        # x -> fp16, layout [j%128, jc, b, d]
        x16 = const.tile([128, JC, B, D], F16)
        for b in range(B):
            nc.gpsimd.dma_start(
                out=x16[:, :, b, :],
                in_=x[b].rearrange("(c p) d -> p c d", p=128))

        # GRU weights: load [192,64] as two partition blocks, PE-transpose gates
        wih_sb = const.tile([128, 2, D], F32)
        nc.sync.dma_start(wih_sb[:, 0, :], w_ih[0:128, :])
        nc.sync.dma_start(wih_sb[0:64, 1, :], w_ih[128:192, :])
        whh_sb = const.tile([128, 2, H], F32)
        nc.sync.dma_start(whh_sb[:, 0, :], w_hh[0:128, :])
        nc.sync.dma_start(whh_sb[0:64, 1, :], w_hh[128:192, :])
        # wT[:, 0:3] = w_ih^T gates r,z,n ; wT[:, 3:6] = w_hh^T
        wT = const.tile([64, 6, 64], F32R)
        for gi, (src, blk, prow) in enumerate([
                (wih_sb, 0, 0), (wih_sb, 0, 64), (wih_sb, 1, 0),
                (whh_sb, 0, 0), (whh_sb, 0, 64), (whh_sb, 1, 0)]):
            ps_w = psum_a.tile([64, 64], F32, tag="acc")
            nc.tensor.transpose(ps_w, src[prow:prow + 64, blk, :],
                                ident[prow:prow + 64, prow:prow + 64])
            nc.vector.tensor_copy(wT[:, gi, :], ps_w)

        # biases as [64, 3] (partition = gate-internal dim)
        bsum = const.tile([64, 3], F32)
        bih_sb = const.tile([64, 3], F32)
        nc.sync.dma_start(bih_sb, b_ih.rearrange("(g p) -> p g", p=64))
        bhh_sb = const.tile([64, 3], F32)
        nc.sync.dma_start(bhh_sb, b_hh.rearrange("(g p) -> p g", p=64))
        nc.vector.tensor_add(bsum, bih_sb, bhh_sb)

        gam_sb = const.tile([128, H], F32)
        g_ap = gamma[:]
        nc.gpsimd.dma_start(out=gam_sb, in_=bass.AP(
            tensor=g_ap.tensor, offset=g_ap.offset, ap=[[0, 128]] + list(g_ap.ap)))
        bet_sb = const.tile([128, H], F32)
        b_ap = beta[:]
        nc.gpsimd.dma_start(out=bet_sb, in_=bass.AP(
            tensor=b_ap.tensor, offset=b_ap.offset, ap=[[0, 128]] + list(b_ap.ap)))
        eps_sb = const.tile([128, 1], F32)
        nc.vector.memset(eps_sb, LN_EPS)

        # persistent state
        s_run = const.tile([64, B, NS], F32)          # cumsum per b
        hx = const.tile([64, C, B * NS], F32R)        # relu(cumsum) per c
        h_t = const.tile([64, B * NS], F32R)          # GRU hidden
        osum = const.tile([64, B * NS], F32)          # sum over c of h

        # ---------- Phase A: msgT = (adj @ x)^T per (b, c) ----------
        NJ = 4                   # j-chunks per DMA
        JW = N // NJ             # 1024 columns per DMA chunk
        for c in range(C):
            for b in range(B):
                src_bc = adj_s[b, c].rearrange("(q p) j -> p q j", p=128)
                ps_acc = psum_a.tile([64, NS], F32, tag="acc")
                for jd in range(NJ):
                    a_in = adj_pool.tile([128, NS // 128, JW], F16, tag="a_in")
                    nc.gpsimd.dma_start(
                        out=a_in,
                        in_=src_bc[:, :, jd * JW:(jd + 1) * JW])
                    for jl in range(JW // 128):
                        jc = jd * (JW // 128) + jl
                        ps_tr = psum_t.tile([128, NS // 128, 128], F16, tag="tr")
                        for q in range(NS // 128):
                            nc.tensor.transpose(
                                ps_tr[:, q, :],
                                a_in[:, q, bass.ts(jl, 128)], ident16)
                        adjT = adjt_pool.tile([128, NS // 128, 128], F16, tag="adjT")
                        if jc % 2 == 0:
                            nc.vector.tensor_copy(adjT, ps_tr)
                        else:
                            nc.scalar.copy(adjT, ps_tr)
                        nc.tensor.matmul(
                            ps_acc, x16[:, jc, b, :], adjT,
                            start=(jc == 0), stop=(jc == JC - 1))
                # cumsum + relu
                if c == 0:
                    nc.vector.tensor_copy(s_run[:, b, :], ps_acc)
                else:
                    nc.vector.tensor_add(s_run[:, b, :], s_run[:, b, :], ps_acc)
                nc.vector.tensor_relu(
                    hx[:, c, b * NS:(b + 1) * NS], s_run[:, b, :])

                # ---------- Phase B: GRU step c, half b ----------
                half = b
                sl = slice(half * NS, (half + 1) * NS)
                hx_c = hx[:, c, sl]
                ps_r = psum.tile([64, NS], F32, tag="ps_r")
                ps_z = psum.tile([64, NS], F32, tag="ps_z")
                ps_n = psum.tile([64, NS], F32, tag="ps_n")
                nc.tensor.matmul(ps_r, wT[:, 0, :], hx_c,
                                 start=True, stop=(c == 0))
                nc.tensor.matmul(ps_z, wT[:, 1, :], hx_c,
                                 start=True, stop=(c == 0))
                nc.tensor.matmul(ps_n, wT[:, 2, :], hx_c, start=True, stop=True)
                if c > 0:
                    nc.tensor.matmul(ps_r, wT[:, 3, :], h_t[:, sl],
                                     start=False, stop=True)
                    nc.tensor.matmul(ps_z, wT[:, 4, :], h_t[:, sl],
                                     start=False, stop=True)
                    ps_hn = psum.tile([64, NS], F32, tag="ps_hn")
                    nc.tensor.matmul(ps_hn, wT[:, 5, :], h_t[:, sl],
                                     start=True, stop=True)
                r_sb = gru.tile([64, NS], F32, tag="r")
                nc.scalar.activation(r_sb, ps_r, AF.Sigmoid, bias=bsum[:, 0:1])
                z_sb = gru.tile([64, NS], F32, tag="z")
                nc.scalar.activation(z_sb, ps_z, AF.Sigmoid, bias=bsum[:, 1:2])
                n_sb = gru.tile([64, NS], F32, tag="n")
                if c > 0:
                    t0 = gru.tile([64, NS], F32, tag="t0")
                    nc.vector.tensor_scalar_add(t0, ps_hn, bhh_sb[:, 2:3])
                    t1 = gru.tile([64, NS], F32, tag="t1")
                    nc.vector.tensor_mul(t1, r_sb, t0)
                    t2 = gru.tile([64, NS], F32, tag="t2")
                    nc.vector.tensor_add(t2, t1, ps_n)
                    nc.scalar.activation(n_sb, t2, AF.Tanh, bias=bih_sb[:, 2:3])
                else:
                    nc.scalar.activation(n_sb, ps_n, AF.Tanh, bias=bih_sb[:, 2:3])
                # h' = n + z*(h - n)   (c=0: h=0 -> h' = n - z*n)
                t3 = gru.tile([64, NS], F32, tag="t3")
                if c > 0:
                    nc.vector.tensor_sub(t3, h_t[:, sl], n_sb)
                else:
                    nc.vector.tensor_scalar_mul(t3, n_sb, -1.0)
                t4 = gru.tile([64, NS], F32, tag="t4")
                nc.vector.tensor_mul(t4, z_sb, t3)
                nc.vector.tensor_add(h_t[:, sl], n_sb, t4)
                if c == 0:
                    nc.vector.tensor_copy(osum[:, sl], h_t[:, sl])
                else:
                    nc.vector.tensor_add(osum[:, sl], osum[:, sl], h_t[:, sl])

        # ---------- Phase C: LayerNorm + output ----------
        oT = const.tile([128, B * NS // 128, H], F32)
        for blk in range(B * NS // 128):
            ps_o = psum_a.tile([128, 64], F32, tag="acc")
            nc.tensor.transpose(ps_o, osum[:, bass.ts(blk, 128)], ident[0:64, 0:64])
            nc.vector.tensor_copy(oT[:, blk, :], ps_o)
        stats = const.tile([128, B * NS // 128, 6], F32)
        mv = const.tile([128, B * NS // 128, 2], F32)
        rstd = const.tile([128, B * NS // 128, 1], F32)
        out_st = const.tile([128, B * NS // 128, H], F32)
        for blk in range(B * NS // 128):
            nc.vector.bn_stats(stats[:, blk, :], oT[:, blk, :])
            nc.vector.bn_aggr(mv[:, blk, :], stats[:, blk, :])
        for blk in range(B * NS // 128):
            nc.scalar.activation(rstd[:, blk, :], mv[:, blk, 1:2],
                                 AF.Sqrt, bias=eps_sb)
        for blk in range(B * NS // 128):
            nc.vector.reciprocal(rstd[:, blk, :], rstd[:, blk, :])
            xm = gru.tile([128, H], F32, tag="xm")
            nc.vector.tensor_scalar_sub(xm, oT[:, blk, :], mv[:, blk, 0:1])
            nc.vector.tensor_scalar_mul(xm, xm, rstd[:, blk, :])
            nc.vector.tensor_mul(xm, xm, gam_sb)
            nc.vector.tensor_add(out_st[:, blk, :], xm, bet_sb)
        for b in range(B):
            nc.sync.dma_start(
                out_s[b].rearrange("(q p) d -> p q d", p=128),
                out_st[:, b * (NS // 128):(b + 1) * (NS // 128), :])

    nc.compile()
    return nc


_NC_CACHE = None


def _get_nc():
    global _NC_CACHE
    if _NC_CACHE is None:
        _NC_CACHE = build()
    return _NC_CACHE


def run(inputs, **spmd_kwargs):
    nc = _get_nc()
    adj = np.ascontiguousarray(inputs["adj"], dtype=np.float32)
    in_maps = []
    for k in range(NCORES):
        m = {
            "adj_s": np.ascontiguousarray(adj[:, :, k * NS:(k + 1) * NS, :]),
            "x": np.ascontiguousarray(inputs["x"], dtype=np.float32),
            "w_ih": np.ascontiguousarray(inputs["w_ih"], dtype=np.float32),
            "w_hh": np.ascontiguousarray(inputs["w_hh"], dtype=np.float32),
            "b_ih": np.ascontiguousarray(inputs["b_ih"], dtype=np.float32),
            "b_hh": np.ascontiguousarray(inputs["b_hh"], dtype=np.float32),
            "gamma": np.ascontiguousarray(inputs["gamma"], dtype=np.float32),
            "beta": np.ascontiguousarray(inputs["beta"], dtype=np.float32),
        }
        in_maps.append(m)
    res = run_bass_kernel_spmd(nc, in_maps, list(range(NCORES)), **spmd_kwargs)
    out = np.concatenate([res.results[k]["out_s"] for k in range(NCORES)], axis=1)
    return out.astype(np.float32), res


def kernel(**inputs):
    out, _ = run(inputs)
    return out



# revision 2
# speedup vs baseline: 1.8707x; 1.8707x over previous
"""Trainium2 Bass kernel for nn_CoreDiffusion (gnn_message_passing).

Sharding: node dim N=4096 split across 8 cores (NS=512 rows each).

Per core:
  msg[b,c] = adj[b,c,rows,:] @ x[b]     (adj streamed as fp8-e3m4, x fp16)
  hx[c]    = relu(cumsum_c msg)          (fp32, b packed on partitions)
  GRU over c (block-diag f32r matmuls), sum over c, LayerNorm (fp32).

Precision scheme: adj ~ U[0,1) is centered+scaled on host to 16*(adj-0.5)
and quantized to fp8-e3m4 (kills the subnormal band, halves avg error).
The exact correction 0.5*sum_j x[b,j,d] rides the existing ReLU as a
per-partition bias of (c+1)*8*colsum, and the 1/16 descale is folded into
w_ih on host. Both batch entries are packed on PSUM/SBUF partitions
(b=0 -> 0..63, b=1 -> 64..127), halving vector-engine and GRU-matmul work.

No collectives; full output gathered on host.
"""
import numpy as np
import ml_dtypes
from contextlib import ExitStack

import concourse.bass as bass
import concourse.mybir as mybir
import concourse.tile as tile
from concourse import bacc
from concourse.masks import make_identity
from concourse.bass_utils import run_bass_kernel_spmd

F32 = mybir.dt.float32
F32R = mybir.dt.float32r
F16 = mybir.dt.float16
F8E3 = mybir.dt.float8e3
AF = mybir.ActivationFunctionType

B, C, N, D, H = 2, 4, 4096, 64, 64
NCORES = 8
NS = N // NCORES            # 512 node rows per core
JC = N // 128               # 32 contraction chunks
JH = JC // 2                # 16 chunks per adj DMA
ADJ_SCALE = 16.0            # adj stored as ADJ_SCALE*(adj-0.5) in fp8
LN_EPS = 1e-5


def build():
    nc = bacc.Bacc("TRN2", target_bir_lowering=False, debug=False,
                   num_devices=NCORES)
    adj_t = nc.declare_dram_parameter("adj_t", [B, C, N, NS], F8E3,
                                      isOutput=False)
    x_r = nc.declare_dram_parameter("x_r", [B, 128, JC, D], F16,
                                    isOutput=False)
    wT = nc.declare_dram_parameter("wT", [128, 6, 128], F32R, isOutput=False)
    bsum = nc.declare_dram_parameter("bsum", [128, 4], F32, isOutput=False)
    rbias = nc.declare_dram_parameter("rbias", [128, C], F32, isOutput=False)
    gam = nc.declare_dram_parameter("gam", [128, H], F32, isOutput=False)
    bet = nc.declare_dram_parameter("bet", [128, H], F32, isOutput=False)
    out_s = nc.declare_dram_parameter("out_s", [B, NS, H], F32, isOutput=True)

    with tile.TileContext(nc) as tc, ExitStack() as ctx:
        const = ctx.enter_context(tc.tile_pool(name="const", bufs=1))
        adj_pool = ctx.enter_context(tc.tile_pool(name="adj", bufs=3))
        hx_pool = ctx.enter_context(tc.tile_pool(name="hx", bufs=2))
        gru = ctx.enter_context(tc.tile_pool(name="gru", bufs=2))
        psum_m = ctx.enter_context(tc.tile_pool(name="psum_m", bufs=2,
                                                space="PSUM"))
        psum_g = ctx.enter_context(tc.tile_pool(name="psum_g", bufs=1,
                                                space="PSUM"))
        psum_t = ctx.enter_context(tc.tile_pool(name="psum_t", bufs=2,
                                                space="PSUM"))

        # ---------- persistent state / params ----------
        x16 = const.tile([128, B, JC, D], F16)
        wT_sb = const.tile([128, 6, 128], F32R)
        bsum_sb = const.tile([128, 4], F32)
        rbias_sb = const.tile([128, C], F32)
        gam_sb = const.tile([128, H], F32)
        bet_sb = const.tile([128, H], F32)
        eps_sb = const.tile([128, 1], F32)
        ident = const.tile([128, 128], F32)

        s_run = const.tile([128, NS], F32)
        h_t = const.tile([128, NS], F32R)
        osum = const.tile([128, NS], F32)

        # first adj chunk before anything else so PE starts ASAP
        a_t0 = adj_pool.tile([128, JH, NS], F8E3, tag="a")
        nc.sync.dma_start(
            a_t0, adj_t[0, 0, 0:JH * 128, :].rearrange("(j p) n -> p j n",
                                                       p=128))
        for b in range(B):
            nc.sync.dma_start(x16[:, b, :, :], x_r[b])
        nc.sync.dma_start(wT_sb, wT[:, :, :])
        nc.sync.dma_start(bsum_sb, bsum[:, :])
        nc.sync.dma_start(rbias_sb, rbias[:, :])
        nc.sync.dma_start(gam_sb, gam[:, :])
        nc.sync.dma_start(bet_sb, bet[:, :])
        nc.vector.memset(eps_sb, LN_EPS)
        make_identity(nc, ident)

        def gru_step(c, hx_c):
            """GRU step c on [128, NS] (b packed on partitions)."""
            ps_r = psum_g.tile([128, NS], F32, tag="ps_r")
            ps_z = psum_g.tile([128, NS], F32, tag="ps_z")
            ps_n = psum_g.tile([128, NS], F32, tag="ps_n")
            nc.tensor.matmul(ps_r, wT_sb[:, 0, :], hx_c,
                             start=True, stop=(c == 0))
            nc.tensor.matmul(ps_z, wT_sb[:, 1, :], hx_c,
                             start=True, stop=(c == 0))
            nc.tensor.matmul(ps_n, wT_sb[:, 2, :], hx_c, start=True, stop=True)
            if c > 0:
                nc.tensor.matmul(ps_r, wT_sb[:, 3, :], h_t,
                                 start=False, stop=True)
                nc.tensor.matmul(ps_z, wT_sb[:, 4, :], h_t,
                                 start=False, stop=True)
                ps_hn = psum_g.tile([128, NS], F32, tag="ps_hn")
                nc.tensor.matmul(ps_hn, wT_sb[:, 5, :], h_t,
                                 start=True, stop=True)
            r_sb = gru.tile([128, NS], F32, tag="r")
            nc.scalar.activation(r_sb, ps_r, AF.Sigmoid, bias=bsum_sb[:, 0:1])
            z_sb = gru.tile([128, NS], F32, tag="z")
            nc.scalar.activation(z_sb, ps_z, AF.Sigmoid, bias=bsum_sb[:, 1:2])
            n_sb = gru.tile([128, NS], F32, tag="n")
            if c > 0:
                t0 = gru.tile([128, NS], F32, tag="t0")
                nc.vector.tensor_scalar_add(t0, ps_hn, bsum_sb[:, 3:4])
                t1 = gru.tile([128, NS], F32, tag="t1")
                nc.vector.tensor_mul(t1, r_sb, t0)
                t2 = gru.tile([128, NS], F32, tag="t2")
                nc.vector.tensor_add(t2, t1, ps_n)
                nc.scalar.activation(n_sb, t2, AF.Tanh, bias=bsum_sb[:, 2:3])
            else:
                nc.scalar.activation(n_sb, ps_n, AF.Tanh, bias=bsum_sb[:, 2:3])
            # h' = n + z*(h - n)   (c=0: h=0 -> h' = n - z*n)
            t3 = gru.tile([128, NS], F32, tag="t3")
            if c > 0:
                nc.vector.tensor_sub(t3, h_t, n_sb)
            else:
                nc.vector.tensor_scalar_mul(t3, n_sb, -1.0)
            t4 = gru.tile([128, NS], F32, tag="t4")
            nc.vector.tensor_mul(t4, z_sb, t3)
            nc.vector.tensor_add(h_t, n_sb, t4)
            if c == 0:
                nc.vector.tensor_copy(osum, h_t)
            else:
                nc.vector.tensor_add(osum, osum, h_t)

        # ---------- main loop: msg matmuls for c, then GRU for c-1 ----------
        hx_tiles = [None] * C
        for c in range(C):
            ps_msg = psum_m.tile([128, NS], F32, tag="msg")
            for b in range(B):
                for half in range(2):
                    if c == 0 and b == 0 and half == 0:
                        a_t = a_t0
                    else:
                        a_t = adj_pool.tile([128, JH, NS], F8E3, tag="a")
                        r0 = half * JH * 128
                        nc.sync.dma_start(
                            a_t,
                            adj_t[b, c, r0:r0 + JH * 128, :].rearrange(
                                "(j p) n -> p j n", p=128))
                    for j in range(JH):
                        jc = half * JH + j
                        nc.tensor.matmul(
                            ps_msg[64 * b:64 * (b + 1), :],
                            x16[:, b, jc, :], a_t[:, j, :],
                            start=(jc == 0), stop=(jc == JC - 1))
            if c == 0:
                nc.vector.tensor_copy(s_run, ps_msg)
            else:
                nc.vector.tensor_add(s_run, s_run, ps_msg)
            hx_c = hx_pool.tile([128, NS], F32R, tag="hx")
            nc.scalar.activation(hx_c, s_run, AF.Relu,
                                 bias=rbias_sb[:, c:c + 1])
            hx_tiles[c] = hx_c
            if c >= 1:
                gru_step(c - 1, hx_tiles[c - 1])
        gru_step(C - 1, hx_tiles[C - 1])

        # ---------- LayerNorm + output ----------
        # osum[p, n]: p = 64*b + h. Transpose 128x128 blocks -> oT[node, 64*b+h]
        oT = const.tile([128, 4, 128], F32)
        for blk in range(4):
            ps_t = psum_t.tile([128, 128], F32, tag="t")
            nc.tensor.transpose(ps_t, osum[:, 128 * blk:128 * (blk + 1)],
                                ident)
            nc.vector.tensor_copy(oT[:, blk, :], ps_t)
        stats = const.tile([128, 4, 2, 6], F32)
        mv = const.tile([128, 4, 2, 2], F32)
        rstd = const.tile([128, 4, 2, 1], F32)
        out_st = const.tile([128, B, 4, H], F32)
        for blk in range(4):
            for b in range(B):
                nc.vector.bn_stats(stats[:, blk, b, :],
                                   oT[:, blk, 64 * b:64 * (b + 1)])
                nc.vector.bn_aggr(mv[:, blk, b, :], stats[:, blk, b, :])
        nc.scalar.activation(rstd, mv[:, :, :, 1:2], AF.Sqrt, bias=eps_sb)
        nc.vector.reciprocal(rstd, rstd)
        for blk in range(4):
            for b in range(B):
                xm = gru.tile([128, H], F32, tag="xm")
                nc.vector.tensor_scalar_sub(xm, oT[:, blk, 64 * b:64 * (b + 1)],
                                            mv[:, blk, b, 0:1])
                nc.vector.tensor_scalar_mul(xm, xm, rstd[:, blk, b, :])
                nc.vector.tensor_mul(xm, xm, gam_sb)
                nc.vector.tensor_add(out_st[:, b, blk, :], xm, bet_sb)
        for b in range(B):
            nc.sync.dma_start(
                out_s[b].rearrange("(q p) d -> p q d", p=128),
                out_st[:, b, :, :])

    nc.compile()
    return nc


_NC_CACHE = None


def _get_nc():
    global _NC_CACHE
    if _NC_CACHE is None:
        _NC_CACHE = build()
    return _NC_CACHE


def _prep_host(inputs):
    """Host-side layout/precision prep shared by all cores."""
    adj = np.asarray(inputs["adj"], dtype=np.float32)
    x = np.asarray(inputs["x"], dtype=np.float32)
    w_ih = np.asarray(inputs["w_ih"], dtype=np.float32)
    w_hh = np.asarray(inputs["w_hh"], dtype=np.float32)
    b_ih = np.asarray(inputs["b_ih"], dtype=np.float32)
    b_hh = np.asarray(inputs["b_hh"], dtype=np.float32)
    gamma = np.asarray(inputs["gamma"], dtype=np.float32)
    beta = np.asarray(inputs["beta"], dtype=np.float32)

    adj_q = ((adj - 0.5) * ADJ_SCALE).astype(ml_dtypes.float8_e3m4)
    x_r = np.ascontiguousarray(
        x.astype(np.float16).reshape(B, JC, 128, D).transpose(0, 2, 1, 3))

    wT = np.zeros((128, 6, 128), dtype=np.float32)
    for g in range(3):
        wg_ih = (w_ih[g * H:(g + 1) * H, :] / ADJ_SCALE).T  # [D, H]
        wg_hh = w_hh[g * H:(g + 1) * H, :].T                # [H, H]
        for half in range(2):
            s = 64 * half
            wT[s:s + 64, g, s:s + 64] = wg_ih
            wT[s:s + 64, g + 3, s:s + 64] = wg_hh

    bsum = np.zeros((128, 4), dtype=np.float32)
    for half in range(2):
        s = 64 * half
        bsum[s:s + 64, 0] = b_ih[0:H] + b_hh[0:H]
        bsum[s:s + 64, 1] = b_ih[H:2 * H] + b_hh[H:2 * H]
        bsum[s:s + 64, 2] = b_ih[2 * H:3 * H]
        bsum[s:s + 64, 3] = b_hh[2 * H:3 * H]

    colsum = x.sum(axis=1)  # [B, D] exact fp32
    rbias = np.zeros((128, C), dtype=np.float32)
    for b in range(B):
        for c in range(C):
            rbias[64 * b:64 * (b + 1), c] = \
                (c + 1) * 0.5 * ADJ_SCALE * colsum[b]

    gam = np.ascontiguousarray(np.broadcast_to(gamma, (128, H)),
                               dtype=np.float32)
    bet = np.ascontiguousarray(np.broadcast_to(beta, (128, H)),
                               dtype=np.float32)
    return adj_q, x_r, wT, bsum, rbias, gam, bet


def run(inputs, **spmd_kwargs):
    nc = _get_nc()
    adj_q, x_r, wT, bsum, rbias, gam, bet = _prep_host(inputs)
    in_maps = []
    for k in range(NCORES):
        rows = slice(k * NS, (k + 1) * NS)
        m = {
            "adj_t": np.ascontiguousarray(
                adj_q[:, :, rows, :].transpose(0, 1, 3, 2)),
            "x_r": x_r,
            "wT": wT,
            "bsum": bsum,
            "rbias": rbias,
            "gam": gam,
            "bet": bet,
        }
        in_maps.append(m)
    res = run_bass_kernel_spmd(nc, in_maps, list(range(NCORES)), **spmd_kwargs)
    out = np.concatenate([res.results[k]["out_s"] for k in range(NCORES)],
                         axis=1)
    return out.astype(np.float32), res


def kernel(**inputs):
    out, _ = run(inputs)
    return out


# revision 5
# speedup vs baseline: 2.1406x; 1.1443x over previous
"""Trainium2 Bass kernel for nn_CoreDiffusion (gnn_message_passing).

Sharding: node dim N=4096 split across 8 cores (NS=512 rows each).

Per core:
  msg[b,c] = adj[b,c,rows,:] @ x[b]     (adj streamed as fp8-e3m4, x fp16)
  hx[c]    = relu(cumsum_c msg)          (fp32 accum, b packed on partitions)
  GRU over c (fp16 matmuls + elementwise), sum over c, LayerNorm.

Precision scheme: adj ~ U[0,1) is centered+scaled on host to 16*(adj-0.5)
and quantized to fp8-e3m4 (kills the subnormal band, halves avg error).
The exact correction 0.5*sum_j x[b,j,d] rides the existing ReLU as a
per-partition bias of (c+1)*8*colsum, and the 1/16 descale is folded into
w_ih on host. Both batch entries are packed on PSUM/SBUF partitions
(b=0 -> 0..63, b=1 -> 64..127), halving vector-engine and GRU-matmul work.

No collectives; full output gathered on host.
"""
import numpy as np
import ml_dtypes
from contextlib import ExitStack

import concourse.bass as bass
import concourse.mybir as mybir
import concourse.tile as tile
from concourse import bacc
from concourse.masks import make_identity
from concourse.bass_utils import run_bass_kernel_spmd

F32 = mybir.dt.float32
F32R = mybir.dt.float32r
F16 = mybir.dt.float16
F8E3 = mybir.dt.float8e3
AF = mybir.ActivationFunctionType

B, C, N, D, H = 2, 4, 4096, 64, 64
NCORES = 8
NS = N // NCORES            # 512 node rows per core
JC = N // 128               # 32 contraction chunks
JH = JC // 2                # 16 chunks per adj DMA
ADJ_SCALE = 16.0            # adj stored as ADJ_SCALE*(adj-0.5) in fp8
LN_EPS = 1e-5


def build():
    nc = bacc.Bacc("TRN2", target_bir_lowering=False, debug=False,
                   num_devices=NCORES)
    adj_t = nc.declare_dram_parameter("adj_t", [B, C, N, NS], F8E3,
                                      isOutput=False)
    x_r = nc.declare_dram_parameter("x_r", [B, 128, JC, D], F16,
                                    isOutput=False)
    wT = nc.declare_dram_parameter("wT", [128, 6, 128], F16, isOutput=False)
    bsum = nc.declare_dram_parameter("bsum", [128, 4], F32, isOutput=False)
    rbias = nc.declare_dram_parameter("rbias", [128, C], F32, isOutput=False)
    gam = nc.declare_dram_parameter("gam", [128, H], F16, isOutput=False)
    bet = nc.declare_dram_parameter("bet", [128, H], F16, isOutput=False)
    out_s = nc.declare_dram_parameter("out_s", [128, B, 4, H], F32,
                                      isOutput=True)

    with tile.TileContext(nc) as tc, ExitStack() as ctx:
        ctx.enter_context(nc.allow_low_precision(
            reason="fp16 GRU/LN elementwise; rel tolerance 2e-2"))
        const = ctx.enter_context(tc.tile_pool(name="const", bufs=1))
        adj_pool = ctx.enter_context(tc.tile_pool(name="adj", bufs=3))
        hx_pool = ctx.enter_context(tc.tile_pool(name="hx", bufs=2))
        gru = ctx.enter_context(tc.tile_pool(name="gru", bufs=2))
        psum_m = ctx.enter_context(tc.tile_pool(name="psum_m", bufs=2,
                                                space="PSUM"))
        psum_g = ctx.enter_context(tc.tile_pool(name="psum_g", bufs=1,
                                                space="PSUM"))
        psum_t = ctx.enter_context(tc.tile_pool(name="psum_t", bufs=2,
                                                space="PSUM"))

        # ---------- persistent state / params ----------
        x16 = const.tile([128, B, JC, D], F16)
        wT_sb = const.tile([128, 6, 128], F16)
        bsum_sb = const.tile([128, 4], F32)
        rbias_sb = const.tile([128, C], F32)
        gam_sb = const.tile([128, H], F16)
        bet_sb = const.tile([128, H], F16)
        eps_sb = const.tile([128, 1], F32)
        scr_sb = const.tile([128, 1], F16)
        ident = const.tile([128, 128], F32)
        ident16 = const.tile([128, 128], F16)

        s_run = const.tile([128, NS], F32)
        h_t = const.tile([128, NS], F16)
        osum = const.tile([128, NS], F16)

        # DMA order: x(b=0) first, then first adj chunk, so PE starts ASAP.
        nc.sync.dma_start(x16[:, 0, :, :], x_r[0])
        a_t0 = adj_pool.tile([128, JH, NS], F8E3, tag="a")
        nc.sync.dma_start(
            a_t0, adj_t[0, 0, 0:JH * 128, :].rearrange("(j p) n -> p j n",
                                                       p=128))
        nc.sync.dma_start(x16[:, 1, :, :], x_r[1])
        nc.sync.dma_start(wT_sb, wT[:, :, :])
        nc.sync.dma_start(bsum_sb, bsum[:, :])
        nc.sync.dma_start(rbias_sb, rbias[:, :])
        nc.sync.dma_start(gam_sb, gam[:, :])
        nc.sync.dma_start(bet_sb, bet[:, :])
        # preload activation tables off the critical path
        nc.vector.memset(eps_sb, LN_EPS)
        nc.scalar.activation(scr_sb, eps_sb, AF.Relu)
        nc.scalar.activation(scr_sb, eps_sb, AF.Sigmoid)
        nc.scalar.activation(scr_sb, eps_sb, AF.Tanh)
        nc.scalar.activation(scr_sb, eps_sb, AF.Sqrt)
        make_identity(nc, ident)
        nc.vector.tensor_copy(ident16, ident)

        def gru_step(c, hx_c):
            """GRU step c on [128, NS] (b packed on partitions)."""
            ps_r = psum_g.tile([128, NS], F32, tag="ps_r")
            ps_z = psum_g.tile([128, NS], F32, tag="ps_z")
            ps_n = psum_g.tile([128, NS], F32, tag="ps_n")
            nc.tensor.matmul(ps_r, wT_sb[:, 0, :], hx_c,
                             start=True, stop=(c == 0))
            nc.tensor.matmul(ps_z, wT_sb[:, 1, :], hx_c,
                             start=True, stop=(c == 0))
            nc.tensor.matmul(ps_n, wT_sb[:, 2, :], hx_c, start=True, stop=True)
            if c > 0:
                nc.tensor.matmul(ps_r, wT_sb[:, 3, :], h_t,
                                 start=False, stop=True)
                nc.tensor.matmul(ps_z, wT_sb[:, 4, :], h_t,
                                 start=False, stop=True)
                ps_hn = psum_g.tile([128, NS], F32, tag="ps_hn")
                nc.tensor.matmul(ps_hn, wT_sb[:, 5, :], h_t,
                                 start=True, stop=True)
            r_sb = gru.tile([128, NS], F16, tag="r")
            nc.scalar.activation(r_sb, ps_r, AF.Sigmoid, bias=bsum_sb[:, 0:1])
            z_sb = gru.tile([128, NS], F16, tag="z")
            nc.scalar.activation(z_sb, ps_z, AF.Sigmoid, bias=bsum_sb[:, 1:2])
            n_sb = gru.tile([128, NS], F16, tag="n")
            if c > 0:
                t0 = gru.tile([128, NS], F16, tag="t0")
                nc.vector.tensor_scalar_add(t0, ps_hn, bsum_sb[:, 3:4])
                t1 = gru.tile([128, NS], F16, tag="t1")
                nc.vector.tensor_mul(t1, r_sb, t0)
                t2 = gru.tile([128, NS], F32, tag="t2")
                nc.vector.tensor_add(t2, t1, ps_n)
                nc.scalar.activation(n_sb, t2, AF.Tanh, bias=bsum_sb[:, 2:3])
            else:
                nc.scalar.activation(n_sb, ps_n, AF.Tanh, bias=bsum_sb[:, 2:3])
            # h' = n + z*(h - n)   (c=0: h=0 -> h' = n - z*n)
            t3 = gru.tile([128, NS], F16, tag="t3")
            if c > 0:
                nc.vector.tensor_sub(t3, h_t, n_sb)
            else:
                nc.vector.tensor_scalar_mul(t3, n_sb, -1.0)
            t4 = gru.tile([128, NS], F16, tag="t4")
            nc.vector.tensor_mul(t4, z_sb, t3)
            nc.vector.tensor_add(h_t, n_sb, t4)
            if c == 0:
                nc.vector.tensor_copy(osum, h_t)
            else:
                nc.vector.tensor_add(osum, osum, h_t)

        # ---------- main loop ----------
        # GRU step c-1 is emitted midway through msg-c's matmuls so its
        # ACT/DVE chain overlaps the remaining PE work (incl. for c = C-1).
        hx_tiles = [None] * C
        for c in range(C):
            ps_msg = psum_m.tile([128, NS], F32, tag="msg")
            for b in range(B):
                for half in range(2):
                    if c == 0 and b == 0 and half == 0:
                        a_t = a_t0
                    else:
                        a_t = adj_pool.tile([128, JH, NS], F8E3, tag="a")
                        r0 = half * JH * 128
                        nc.sync.dma_start(
                            a_t,
                            adj_t[b, c, r0:r0 + JH * 128, :].rearrange(
                                "(j p) n -> p j n", p=128))
                    for j in range(JH):
                        jc = half * JH + j
                        nc.tensor.matmul(
                            ps_msg[64 * b:64 * (b + 1), :],
                            x16[:, b, jc, :], a_t[:, j, :],
                            start=(jc == 0), stop=(jc == JC - 1))
                if b == 0 and c >= 1:
                    gru_step(c - 1, hx_tiles[c - 1])
            if c == 0:
                nc.vector.tensor_copy(s_run, ps_msg)
            else:
                nc.vector.tensor_add(s_run, s_run, ps_msg)
            hx_c = hx_pool.tile([128, NS], F16, tag="hx")
            nc.scalar.activation(hx_c, s_run, AF.Relu,
                                 bias=rbias_sb[:, c:c + 1])
            hx_tiles[c] = hx_c
        gru_step(C - 1, hx_tiles[C - 1])

        # ---------- LayerNorm + output ----------
        # osum[p, n]: p = 64*b + h. Transpose 128x128 blocks -> oT[node, 64*b+h]
        oT = const.tile([128, 4, 128], F16)
        for blk in range(4):
            ps_t = psum_t.tile([128, 128], F16, tag="t")
            nc.tensor.transpose(ps_t, osum[:, 128 * blk:128 * (blk + 1)],
                                ident16)
            nc.vector.tensor_copy(oT[:, blk, :], ps_t)
        stats = const.tile([128, 4, 2, 6], F32)
        mv = const.tile([128, 4, 2, 2], F32)
        rstd = const.tile([128, 4, 2, 1], F32)
        out_st = const.tile([128, B, 4, H], F32)
        for blk in range(4):
            for b in range(B):
                nc.vector.bn_stats(stats[:, blk, b, :],
                                   oT[:, blk, 64 * b:64 * (b + 1)])
                nc.vector.bn_aggr(mv[:, blk, b, :], stats[:, blk, b, :])
        nc.scalar.activation(rstd, mv[:, :, :, 1:2], AF.Sqrt, bias=eps_sb)
        nc.vector.reciprocal(rstd, rstd)
        for b in range(B):
            for blk in range(4):
                xm = gru.tile([128, H], F16, tag="xm")
                nc.vector.tensor_scalar_sub(xm, oT[:, blk, 64 * b:64 * (b + 1)],
                                            mv[:, blk, b, 0:1])
                nc.vector.tensor_scalar_mul(xm, xm, rstd[:, blk, b, :])
                nc.vector.tensor_mul(xm, xm, gam_sb)
                nc.vector.tensor_add(out_st[:, b, blk, :], xm, bet_sb)
            nc.sync.dma_start(out_s[:, b, :, :], out_st[:, b, :, :])

    nc.compile()
    return nc


_NC_CACHE = None


def _get_nc():
    global _NC_CACHE
    if _NC_CACHE is None:
        _NC_CACHE = build()
    return _NC_CACHE


def _prep_host(inputs):
    """Host-side layout/precision prep shared by all cores."""
    adj = np.asarray(inputs["adj"], dtype=np.float32)
    x = np.asarray(inputs["x"], dtype=np.float32)
    w_ih = np.asarray(inputs["w_ih"], dtype=np.float32)
    w_hh = np.asarray(inputs["w_hh"], dtype=np.float32)
    b_ih = np.asarray(inputs["b_ih"], dtype=np.float32)
    b_hh = np.asarray(inputs["b_hh"], dtype=np.float32)
    gamma = np.asarray(inputs["gamma"], dtype=np.float32)
    beta = np.asarray(inputs["beta"], dtype=np.float32)

    adj_q = ((adj - 0.5) * ADJ_SCALE).astype(ml_dtypes.float8_e3m4)
    x_r = np.ascontiguousarray(
        x.astype(np.float16).reshape(B, JC, 128, D).transpose(0, 2, 1, 3))

    wT = np.zeros((128, 6, 128), dtype=np.float16)
    for g in range(3):
        wg_ih = (w_ih[g * H:(g + 1) * H, :] / ADJ_SCALE).T  # [D, H]
        wg_hh = w_hh[g * H:(g + 1) * H, :].T                # [H, H]
        for half in range(2):
            s = 64 * half
            wT[s:s + 64, g, s:s + 64] = wg_ih
            wT[s:s + 64, g + 3, s:s + 64] = wg_hh

    bsum = np.zeros((128, 4), dtype=np.float32)
    for half in range(2):
        s = 64 * half
        bsum[s:s + 64, 0] = b_ih[0:H] + b_hh[0:H]
        bsum[s:s + 64, 1] = b_ih[H:2 * H] + b_hh[H:2 * H]
        bsum[s:s + 64, 2] = b_ih[2 * H:3 * H]
        bsum[s:s + 64, 3] = b_hh[2 * H:3 * H]

    colsum = x.sum(axis=1)  # [B, D] exact fp32
    rbias = np.zeros((128, C), dtype=np.float32)
    for b in range(B):
        for c in range(C):
            rbias[64 * b:64 * (b + 1), c] = \
                (c + 1) * 0.5 * ADJ_SCALE * colsum[b]

    gam = np.ascontiguousarray(np.broadcast_to(gamma, (128, H)),
                               dtype=np.float16)
    bet = np.ascontiguousarray(np.broadcast_to(beta, (128, H)),
                               dtype=np.float16)
    return adj_q, x_r, wT, bsum, rbias, gam, bet


def run(inputs, **spmd_kwargs):
    nc = _get_nc()
    adj_q, x_r, wT, bsum, rbias, gam, bet = _prep_host(inputs)
    in_maps = []
    for k in range(NCORES):
        rows = slice(k * NS, (k + 1) * NS)
        m = {
            "adj_t": np.ascontiguousarray(
                adj_q[:, :, rows, :].transpose(0, 1, 3, 2)),
            "x_r": x_r,
            "wT": wT,
            "bsum": bsum,
            "rbias": rbias,
            "gam": gam,
            "bet": bet,
        }
        in_maps.append(m)
    res = run_bass_kernel_spmd(nc, in_maps, list(range(NCORES)), **spmd_kwargs)
    # out_s[p, b, q, h] -> out[b, q*128 + p, h]
    out = np.concatenate(
        [res.results[k]["out_s"].transpose(1, 2, 0, 3).reshape(B, NS, H)
         for k in range(NCORES)], axis=1)
    return out.astype(np.float32), res


def kernel(**inputs):
    out, _ = run(inputs)
    return out


# revision 12
# speedup vs baseline: 2.3964x; 1.1195x over previous
"""Trainium2 Bass kernel for nn_CoreDiffusion (gnn_message_passing).

Sharding: node dim N=4096 split across 8 cores (NS=512 rows each).

Per core:
  msg[b,c] = adj[b,c,rows,:] @ x[b]     (adj streamed as fp8-e3m4, x fp16)
  hx[c]    = relu(cumsum_c msg)          (fp32 accum, b packed on partitions)
  GRU over c (fp16 matmuls + elementwise), sum over c, LayerNorm.

Precision scheme: adj ~ U[0,1) is centered+scaled on host to 16*(adj-0.5)
and quantized to fp8-e3m4 (kills the subnormal band, halves avg error).
The exact correction 0.5*sum_j x[b,j,d] rides the existing ReLU as a
per-partition bias of (c+1)*8*colsum, and the 1/16 descale is folded into
w_ih on host. Both batch entries are packed on PSUM/SBUF partitions
(b=0 -> 0..63, b=1 -> 64..127), halving vector-engine and GRU-matmul work.

No collectives; full output gathered on host.
"""
import numpy as np
import ml_dtypes
from contextlib import ExitStack

import concourse.bass as bass
import concourse.mybir as mybir
import concourse.tile as tile
from concourse import bacc
from concourse.masks import make_identity
from concourse.bass_utils import run_bass_kernel_spmd

F32 = mybir.dt.float32
F32R = mybir.dt.float32r
F16 = mybir.dt.float16
F8E3 = mybir.dt.float8e3
AF = mybir.ActivationFunctionType

B, C, N, D, H = 2, 4, 4096, 64, 64
NCORES = 8
NS = N // NCORES            # 512 node rows per core
JC = N // 128               # 32 contraction chunks
JH = JC // 2                # 16 chunks per adj DMA
ADJ_SCALE = 16.0            # adj stored as ADJ_SCALE*(adj-0.5) in fp8
LN_EPS = 1e-5


def build():
    nc = bacc.Bacc("TRN2", target_bir_lowering=False, debug=False,
                   num_devices=NCORES)
    adj_t = nc.declare_dram_parameter("adj_t", [B, C, N, NS], F8E3,
                                      isOutput=False)
    x_r = nc.declare_dram_parameter("x_r", [B, 128, JC, D], F16,
                                    isOutput=False)
    wT = nc.declare_dram_parameter("wT", [128, 6, 128], F16, isOutput=False)
    bsum = nc.declare_dram_parameter("bsum", [128, 4], F32, isOutput=False)
    rbias = nc.declare_dram_parameter("rbias", [128, C], F32, isOutput=False)
    gam = nc.declare_dram_parameter("gam", [128, H], F16, isOutput=False)
    bet = nc.declare_dram_parameter("bet", [128, H], F16, isOutput=False)
    out_s = nc.declare_dram_parameter("out_s", [128, B, 4, H], F32,
                                      isOutput=True)

    with tile.TileContext(nc) as tc, ExitStack() as ctx:
        ctx.enter_context(nc.allow_low_precision(
            reason="fp16 GRU/LN elementwise; rel tolerance 2e-2"))
        const = ctx.enter_context(tc.tile_pool(name="const", bufs=1))
        adj_pool = ctx.enter_context(tc.tile_pool(name="adj", bufs=4))
        hx_pool = ctx.enter_context(tc.tile_pool(name="hx", bufs=2))
        gru = ctx.enter_context(tc.tile_pool(name="gru", bufs=2))
        psum_m = ctx.enter_context(tc.tile_pool(name="psum_m", bufs=2,
                                                space="PSUM"))
        psum_h = ctx.enter_context(tc.tile_pool(name="psum_h", bufs=1,
                                                space="PSUM"))
        psum_g = ctx.enter_context(tc.tile_pool(name="psum_g", bufs=1,
                                                space="PSUM"))
        psum_t = ctx.enter_context(tc.tile_pool(name="psum_t", bufs=1,
                                                space="PSUM"))

        # ---------- persistent state / params ----------
        x16 = const.tile([128, B, JC, D], F16)
        wT_sb = const.tile([128, 6, 128], F16)
        bsum_sb = const.tile([128, 4], F32)
        rbias_sb = const.tile([128, C], F32)
        gam_sb = const.tile([128, H], F16)
        bet_sb = const.tile([128, H], F16)
        eps_sb = const.tile([128, 1], F32)
        scr_sb = const.tile([128, 1], F16)
        ident = const.tile([128, 128], F32)
        ident16 = const.tile([128, 128], F16)

        s_run = const.tile([128, NS], F32)
        h_t = const.tile([128, NS], F16)
        osum = const.tile([128, NS], F16)

        # DMA order: x(b=0) first, then first adj chunks, so PE starts ASAP.
        nc.sync.dma_start(x16[:, 0, :, :], x_r[0])
        a_first = []
        for half in range(2):
            a_t = adj_pool.tile([128, JH, NS], F8E3, tag="a")
            r0 = half * JH * 128
            nc.sync.dma_start(
                a_t, adj_t[0, 0, r0:r0 + JH * 128, :].rearrange(
                    "(j p) n -> p j n", p=128))
            a_first.append(a_t)
        nc.sync.dma_start(x16[:, 1, :, :], x_r[1])
        nc.sync.dma_start(wT_sb, wT[:, :, :])
        nc.sync.dma_start(bsum_sb, bsum[:, :])
        nc.sync.dma_start(rbias_sb, rbias[:, :])
        nc.sync.dma_start(gam_sb, gam[:, :])
        nc.sync.dma_start(bet_sb, bet[:, :])
        # preload the sigmoid table (covers relu/sigmoid/tanh/copy) early
        nc.vector.memset(eps_sb, LN_EPS)
        nc.scalar.activation(scr_sb, eps_sb, AF.Sigmoid)
        make_identity(nc, ident)
        nc.vector.tensor_copy(ident16, ident)

        def gru_step(c, hx_c):
            """GRU step c on [128, NS] (b packed on partitions)."""
            ps_r = psum_g.tile([128, NS], F32, tag="ps_r")
            ps_z = psum_g.tile([128, NS], F32, tag="ps_z")
            ps_n = psum_g.tile([128, NS], F32, tag="ps_n")
            nc.tensor.matmul(ps_r, wT_sb[:, 0, :], hx_c,
                             start=True, stop=(c == 0))
            nc.tensor.matmul(ps_z, wT_sb[:, 1, :], hx_c,
                             start=True, stop=(c == 0))
            nc.tensor.matmul(ps_n, wT_sb[:, 2, :], hx_c, start=True, stop=True)
            if c > 0:
                nc.tensor.matmul(ps_r, wT_sb[:, 3, :], h_t,
                                 start=False, stop=True)
                nc.tensor.matmul(ps_z, wT_sb[:, 4, :], h_t,
                                 start=False, stop=True)
                ps_hn = psum_g.tile([128, NS], F32, tag="ps_hn")
                nc.tensor.matmul(ps_hn, wT_sb[:, 5, :], h_t,
                                 start=True, stop=True)
            r_sb = gru.tile([128, NS], F16, tag="r")
            nc.scalar.activation(r_sb, ps_r, AF.Sigmoid, bias=bsum_sb[:, 0:1])
            z_sb = gru.tile([128, NS], F16, tag="z")
            nc.scalar.activation(z_sb, ps_z, AF.Sigmoid, bias=bsum_sb[:, 1:2])
            n_sb = gru.tile([128, NS], F16, tag="n")
            if c > 0:
                t0 = gru.tile([128, NS], F16, tag="t0")
                nc.vector.tensor_scalar_add(t0, ps_hn, bsum_sb[:, 3:4])
                t1 = gru.tile([128, NS], F16, tag="t1")
                nc.vector.tensor_mul(t1, r_sb, t0)
                t2 = gru.tile([128, NS], F32, tag="t2")
                nc.vector.tensor_add(t2, t1, ps_n)
                nc.scalar.activation(n_sb, t2, AF.Tanh, bias=bsum_sb[:, 2:3])
            else:
                nc.scalar.activation(n_sb, ps_n, AF.Tanh, bias=bsum_sb[:, 2:3])
            # h' = n + z*(h - n)   (c=0: h=0 -> h' = n - z*n)
            t3 = gru.tile([128, NS], F16, tag="t3")
            if c > 0:
                nc.vector.tensor_sub(t3, h_t, n_sb)
            else:
                nc.vector.tensor_scalar_mul(t3, n_sb, -1.0)
            t4 = gru.tile([128, NS], F16, tag="t4")
            nc.vector.tensor_mul(t4, z_sb, t3)
            nc.vector.tensor_add(h_t, n_sb, t4)
            if c == 0:
                nc.vector.tensor_copy(osum, h_t)
            else:
                nc.vector.tensor_add(osum, osum, h_t)

        # ---------- main loop ----------
        # msg matmuls use adj as the stationary operand: out[node, d] with
        # nodes on PSUM partitions (free dim = D=64, the cheap direction).
        # A per-(b,c) fp16 copy + 4 PE transposes restore the [d, node]
        # layout the GRU needs. GRU step c-1 is emitted midway through
        # msg-c so its ACT/DVE chain overlaps remaining PE/DMA work.
        hx_tiles = [None] * C
        for c in range(C):
            hxT_ps = psum_h.tile([128, 4, 128], F16, tag="hxT")
            msg16 = [None, None]
            for b in range(B):
                ps_b = psum_m.tile([128, 4, D], F32, tag="pb")
                halves = []
                for half in range(2):
                    if c == 0 and b == 0:
                        halves.append(a_first[half])
                    else:
                        a_t = adj_pool.tile([128, JH, NS], F8E3, tag="a")
                        r0 = half * JH * 128
                        nc.sync.dma_start(
                            a_t,
                            adj_t[b, c, r0:r0 + JH * 128, :].rearrange(
                                "(j p) n -> p j n", p=128))
                        halves.append(a_t)
                # accumulation groups must not interleave within a PSUM
                # bank: run each nb group's full contraction consecutively
                for nb in range(4):
                    for jc in range(JC):
                        a_t = halves[jc // JH]
                        nc.tensor.matmul(
                            ps_b[:, nb, :],
                            a_t[:, jc % JH, 128 * nb:128 * (nb + 1)],
                            x16[:, b, jc, :],
                            start=(jc == 0), stop=(jc == JC - 1))
                m16 = gru.tile([128, 4, D], F16, tag=f"m{b}")
                nc.vector.tensor_copy(m16, ps_b)
                msg16[b] = m16
                if b == 0 and c >= 1:
                    gru_step(c - 1, hx_tiles[c - 1])
            for b in range(B):
                for nb in range(4):
                    nc.tensor.transpose(hxT_ps[64 * b:64 * (b + 1), nb, :],
                                        msg16[b][:, nb, :], ident16)
            if c == 0:
                nc.vector.tensor_copy(s_run, hxT_ps)
            else:
                nc.vector.tensor_add(s_run, s_run, hxT_ps)
            hx_c = hx_pool.tile([128, NS], F16, tag="hx")
            nc.scalar.activation(hx_c, s_run, AF.Relu,
                                 bias=rbias_sb[:, c:c + 1])
            hx_tiles[c] = hx_c
        gru_step(C - 1, hx_tiles[C - 1])

        # ---------- LayerNorm + output ----------
        # osum[p, n]: p = 64*b + h. Transpose 128x128 blocks; stats and
        # normalize read the PSUM result directly (no SBUF staging copy);
        # (x-mu)*rstd runs on ACT as Copy(scale*x + bias).
        ps_ln = psum_t.tile([128, 4, 128], F16, tag="ln")
        for blk in range(4):
            nc.tensor.transpose(ps_ln[:, blk, :],
                                osum[:, 128 * blk:128 * (blk + 1)], ident16)
        stats = const.tile([128, 4, 2, 6], F32)
        mv = const.tile([128, 4, 2, 2], F32)
        rstd = const.tile([128, 4, 2, 1], F32)
        nmr = const.tile([128, 4, 2, 1], F32)
        out_st = const.tile([128, B, 4, H], F32)
        for blk in range(4):
            for b in range(B):
                nc.vector.bn_stats(stats[:, blk, b, :],
                                   ps_ln[:, blk, 64 * b:64 * (b + 1)])
                nc.vector.bn_aggr(mv[:, blk, b, :], stats[:, blk, b, :])
        nc.scalar.activation(rstd, mv[:, :, :, 1:2], AF.Sqrt, bias=eps_sb)
        nc.vector.reciprocal(rstd, rstd)
        nc.vector.tensor_mul(nmr, mv[:, :, :, 0:1], rstd)
        nc.vector.tensor_scalar_mul(nmr, nmr, -1.0)
        for b in range(B):
            for blk in range(4):
                xm = gru.tile([128, H], F16, tag="xm")
                nc.scalar.activation(xm, ps_ln[:, blk, 64 * b:64 * (b + 1)],
                                     AF.Identity, bias=nmr[:, blk, b, :],
                                     scale=rstd[:, blk, b, :])
                xg = gru.tile([128, H], F16, tag="xg")
                nc.vector.tensor_mul(xg, xm, gam_sb)
                nc.vector.tensor_add(out_st[:, b, blk, :], xg, bet_sb)
            nc.sync.dma_start(out_s[:, b, :, :], out_st[:, b, :, :])

    nc.compile()
    return nc


_NC_CACHE = None


def _get_nc():
    global _NC_CACHE
    if _NC_CACHE is None:
        _NC_CACHE = build()
    return _NC_CACHE


def _prep_host(inputs):
    """Host-side layout/precision prep shared by all cores."""
    adj = np.asarray(inputs["adj"], dtype=np.float32)
    x = np.asarray(inputs["x"], dtype=np.float32)
    w_ih = np.asarray(inputs["w_ih"], dtype=np.float32)
    w_hh = np.asarray(inputs["w_hh"], dtype=np.float32)
    b_ih = np.asarray(inputs["b_ih"], dtype=np.float32)
    b_hh = np.asarray(inputs["b_hh"], dtype=np.float32)
    gamma = np.asarray(inputs["gamma"], dtype=np.float32)
    beta = np.asarray(inputs["beta"], dtype=np.float32)

    adj_q = ((adj - 0.5) * ADJ_SCALE).astype(ml_dtypes.float8_e3m4)
    x_r = np.ascontiguousarray(
        x.astype(np.float16).reshape(B, JC, 128, D).transpose(0, 2, 1, 3))

    wT = np.zeros((128, 6, 128), dtype=np.float16)
    for g in range(3):
        wg_ih = (w_ih[g * H:(g + 1) * H, :] / ADJ_SCALE).T  # [D, H]
        wg_hh = w_hh[g * H:(g + 1) * H, :].T                # [H, H]
        for half in range(2):
            s = 64 * half
            wT[s:s + 64, g, s:s + 64] = wg_ih
            wT[s:s + 64, g + 3, s:s + 64] = wg_hh

    bsum = np.zeros((128, 4), dtype=np.float32)
    for half in range(2):
        s = 64 * half
        bsum[s:s + 64, 0] = b_ih[0:H] + b_hh[0:H]
        bsum[s:s + 64, 1] = b_ih[H:2 * H] + b_hh[H:2 * H]
        bsum[s:s + 64, 2] = b_ih[2 * H:3 * H]
        bsum[s:s + 64, 3] = b_hh[2 * H:3 * H]

    colsum = x.sum(axis=1)  # [B, D] exact fp32
    rbias = np.zeros((128, C), dtype=np.float32)
    for b in range(B):
        for c in range(C):
            rbias[64 * b:64 * (b + 1), c] = \
                (c + 1) * 0.5 * ADJ_SCALE * colsum[b]

    gam = np.ascontiguousarray(np.broadcast_to(gamma, (128, H)),
                               dtype=np.float16)
    bet = np.ascontiguousarray(np.broadcast_to(beta, (128, H)),
                               dtype=np.float16)
    return adj_q, x_r, wT, bsum, rbias, gam, bet


def run(inputs, **spmd_kwargs):
    nc = _get_nc()
    adj_q, x_r, wT, bsum, rbias, gam, bet = _prep_host(inputs)
    in_maps = []
    for k in range(NCORES):
        rows = slice(k * NS, (k + 1) * NS)
        m = {
            "adj_t": np.ascontiguousarray(
                adj_q[:, :, rows, :].transpose(0, 1, 3, 2)),
            "x_r": x_r,
            "wT": wT,
            "bsum": bsum,
            "rbias": rbias,
            "gam": gam,
            "bet": bet,
        }
        in_maps.append(m)
    res = run_bass_kernel_spmd(nc, in_maps, list(range(NCORES)), **spmd_kwargs)
    # out_s[p, b, q, h] -> out[b, q*128 + p, h]
    out = np.concatenate(
        [res.results[k]["out_s"].transpose(1, 2, 0, 3).reshape(B, NS, H)
         for k in range(NCORES)], axis=1)
    return out.astype(np.float32), res


def kernel(**inputs):
    out, _ = run(inputs)
    return out
